# revision 14
# baseline (speedup 1.0000x reference)
"""Trainium2 Bass kernel for fused MultiHeadAttention + residual + LayerNorm.

Problem: B=2, L=S=2048, D=768, H=12 heads of dim 64, attention with key-padding
mask, output projection + bias, residual with q, LayerNorm(gamma, beta).

Sharding over 8 NeuronCores: data-parallel over batch (2 groups of 4 cores) x
tensor-parallel over heads (3 heads per core).

V3: full fp8e4m3 DoubleRow pipeline. All matmuls (QKV projections, QK^T
scores, attention-value, output projection) run in fp8 DoubleRow perf mode
(0.5 cycles/row, two 128-row k-tiles per pass = 4x fewer PE cycles vs bf16).
Q is pre-scaled by 1/ln2 in the wq weights so score PSUM arrives as
A8*(s/8); softmax exp splits between ScalarE (table exp, f8 out) and DVE
(one-shot Schraudolph writing e4m3 bits as int8). Denominator via a ones
column at V slot 64 -> PSUM partition 64; norm chain: ScalarE denom copy ->
DVE reciprocal_approx_fast -> Pool partition_broadcast -> DVE multiply.
V scaled x32 and W x8 so the f8 O/Z stores stay in normal range (Z carries
x256, removed in the LayerNorm residual add). ReduceScatter in 8 fp8 chunks;
LayerNorm on chunk pairs (Pool handles the elementwise stages, DVE the
bn_stats/rsqrt) with the last two chunks split for a short tail.
"""

import sys

sys.path.insert(0, "/opt/trn_rl_repo")

import ml_dtypes
import numpy as np

import concourse.bass as bass
import concourse.tile as tile
from concourse import bacc, mybir
from concourse.bass_utils import run_bass_kernel_spmd

F32 = mybir.dt.float32
BF16 = mybir.dt.bfloat16
F8 = mybir.dt.float8e4
I8 = mybir.dt.int8
I32 = mybir.dt.int32
DR = mybir.MatmulPerfMode.DoubleRow

D = 768
HD = 64
HPC = 3  # heads per core
HCOLS = HPC * HD  # 192
B, L, S = 2, 2048, 2048
NCORES = 8
GROUPS = [[0, 1, 2, 3], [4, 5, 6, 7]]
NPAIR = 3  # d-chunk DR pairs for projections (768 = 3 * 256)
SCH = S // 128  # 16 s-chunks
LB = 512
NLB = L // LB  # 4
NCHUNK = 8  # output chunk slots (64 rows/core); RS merged per l-block, split for the last
CROWS = L // NCHUNK  # 256
ORows = CROWS // 4  # 64 rows per core per chunk
LN_EPS = 1e-5
MASK_NEG = -1000000.0

# fp8 Schraudolph / scaling constants
A8 = 8.0 / np.log(2.0)  # e4m3 bits per e-fold
B8C = 7 * 8 - 0.46  # e4m3 exponent bias bits - Schraudolph C
M_SHIFT = 3.0  # global logit shift (softmax-invariant), keeps bits < 127
SQ = 1.0 / np.log(2.0)  # = A8/8, folded into wq
VS = 32.0  # V scale (keeps f8 O in normal range)
WS = 8.0  # W scale (keeps f8 W in normal range)
ZS = VS * WS  # Z carries x256

# exp engine split: True -> ScalarE table exp, False -> DVE Schraudolph
EXP_SCALAR = (True, True, True, False, True, False, True, False,
              True, False, True, False, True, False, True, False)  # 9 S / 7 D

_CACHE: dict = {}


def _build():
    nc = bacc.Bacc("TRN2", target_bir_lowering=False, debug=False, num_devices=NCORES)

    qT8 = nc.dram_tensor("qT8", [NPAIR, 128, 2, L], F8, kind="ExternalInput").ap()
    kT8 = nc.dram_tensor("kT8", [NPAIR, 128, 2, S], F8, kind="ExternalInput").ap()
    vT8 = nc.dram_tensor("vT8", [NPAIR, 128, 2, S], F8, kind="ExternalInput").ap()
    wq8 = nc.dram_tensor("wq8", [128, NPAIR, 2, HCOLS], F8, kind="ExternalInput").ap()
    wk8 = nc.dram_tensor("wk8", [128, NPAIR, 2, HCOLS], F8, kind="ExternalInput").ap()
    wv8 = nc.dram_tensor("wv8", [128, NPAIR, 2, HCOLS], F8, kind="ExternalInput").ap()
    wz8 = nc.dram_tensor("wz8", [96, 2, D], F8, kind="ExternalInput").ap()
    qres = nc.dram_tensor("qres", [512, D], BF16, kind="ExternalInput").ap()
    maskT = nc.dram_tensor("maskT", [128, SCH], I32, kind="ExternalInput").ap()
    gamma1 = nc.dram_tensor("gamma1", [1, D], F32, kind="ExternalInput").ap()
    beta1 = nc.dram_tensor("beta1", [1, D], F32, kind="ExternalInput").ap()
    out = nc.dram_tensor("out", [512, D], F32, kind="ExternalOutput").ap()

    AL = mybir.AluOpType
    ACT = mybir.ActivationFunctionType

    with tile.TileContext(nc, num_cores=NCORES) as tc:
        with (
            tc.tile_pool(name="persist", bufs=1) as pp,
            tc.tile_pool(name="dram", bufs=1, space="DRAM") as dram,
        ):
            kch = [pp.tile([128, 2, S], F8, name=f"kch{p}") for p in range(NPAIR)]
            vch = [pp.tile([128, 2, S], F8, name=f"vch{p}") for p in range(NPAIR)]
            qch = [pp.tile([128, 2, L], F8, name=f"qch{p}") for p in range(NPAIR)]
            wq_sb = pp.tile([128, NPAIR, 2, HCOLS], F8)
            wk_sb = pp.tile([128, NPAIR, 2, HCOLS], F8)
            wv_sb = pp.tile([128, NPAIR, 2, HCOLS], F8)
            wz_sb = pp.tile([96, 2, D], F8)
            KT1 = pp.tile([64, 2, S], F8)  # h0 on 0:32, h1 on 32:64; slot=lo/hi
            QT1 = pp.tile([64, 2, L], F8)
            KT2 = pp.tile([32, 2, S], F8)  # h2
            QT2 = pp.tile([32, 2, L], F8)
            V_sb = pp.tile([128, SCH // 2, HPC, 2, 128], F8)  # col 64 = ones, 65: pad
            OT = pp.tile([96, 2, L], F8)  # z contraction layout
            mask_i = pp.tile([128, SCH], I32)
            mask_f = pp.tile([128, SCH], F32)
            asc_bias = pp.tile([128, SCH], F32)
            dve_bias = pp.tile([128, SCH], F32)
            gam_b = pp.tile([128, D], F32)
            bet_b = pp.tile([128, D], F32)

            Z_dram = dram.tile([L, D], F8)
            Zr_dram = dram.tile([512, D], F8)
            sync_in = dram.tile([4, 192], BF16)
            sync_out = dram.tile([1, 192], BF16)

            # early dummy collective: absorbs inter-core rendezvous skew
            nc.gpsimd.collective_compute(
                "ReduceScatter",
                AL.add,
                replica_groups=GROUPS,
                ins=[sync_in[:, :].opt()],
                outs=[sync_out[:, :].opt()],
            )
            # weight / constant loads
            nc.sync.dma_start(out=wq_sb, in_=wq8[:, :, :, :])
            nc.sync.dma_start(out=wk_sb, in_=wk8[:, :, :, :])
            nc.sync.dma_start(out=wv_sb, in_=wv8[:, :, :, :])
            nc.sync.dma_start(out=wz_sb, in_=wz8[:, :, :])
            nc.sync.dma_start(out=mask_i, in_=maskT[:, :])
            nc.sync.dma_start(out=gam_b, in_=gamma1.to_broadcast([128, D]))
            nc.sync.dma_start(out=bet_b, in_=beta1.to_broadcast([128, D]))
            # input loads: k first (scores need full K), then q, then v
            for p in range(NPAIR):
                nc.sync.dma_start(out=kch[p], in_=kT8[p])
            for p in range(NPAIR):
                nc.sync.dma_start(out=qch[p], in_=qT8[p])
            for p in range(NPAIR):
                nc.sync.dma_start(out=vch[p], in_=vT8[p])

            nc.vector.tensor_copy(mask_f, mask_i)
            # ScalarE exp bias: (1-m)*MASK_NEG - M
            nc.scalar.activation(
                asc_bias, mask_f, ACT.Copy,
                bias=float(MASK_NEG - M_SHIFT), scale=-MASK_NEG,
            )
            # DVE Schraudolph bias: A8*((1-m)*MASK_NEG - M) + B8C
            nc.scalar.activation(
                dve_bias, mask_f, ACT.Copy,
                bias=float(A8 * (MASK_NEG - M_SHIFT) + B8C), scale=float(-A8 * MASK_NEG),
            )
            # ones column for softmax denominators (pad cols zeroed)
            nc.gpsimd.memset(V_sb[:, :, :, :, 64:128], 0.0)
            ones_t = pp.tile([128, SCH // 2, HPC, 2, 1], F8)
            nc.gpsimd.memset(ones_t, 1.0)
            nc.vector.tensor_copy(V_sb[:, :, :, :, 64:65], ones_t)

            # ---- Stage A: projections (fp8 DoubleRow) ----
            with tc.tile_pool(name="psp", bufs=1, space="PSUM") as psp:
                def proj01(dst, w_sb, src, lb, drain_eng):
                    lsl = slice(512 * lb, 512 * (lb + 1))
                    ps = psp.tile([128, 512], F32, tag="pA", bufs=3, name="p01")
                    for p in range(NPAIR):
                        nc.tensor.matmul(
                            ps, w_sb[:, p, :, 0:128], src[p][:, :, lsl],
                            start=(p == 0), stop=(p == NPAIR - 1), perf_mode=DR,
                        )
                    for t in range(2):
                        drain_eng(out=dst[:, t, lsl], in_=ps[64 * t : 64 * (t + 1)])

                def proj2(dst, w_sb, src, lb, drain_eng):
                    lsl = slice(512 * lb, 512 * (lb + 1))
                    ps = psp.tile([64, 512], F32, tag="pB", bufs=2, name="p2")
                    for p in range(NPAIR):
                        nc.tensor.matmul(
                            ps, w_sb[:, p, :, 128:192], src[p][:, :, lsl],
                            start=(p == 0), stop=(p == NPAIR - 1), perf_mode=DR,
                        )
                    for t in range(2):
                        drain_eng(out=dst[:, t, lsl], in_=ps[32 * t : 32 * (t + 1)])

                dve_cp = nc.vector.tensor_copy
                sc_cp = nc.scalar.copy

                # warm fillers keyed on the input DMA pairs keep the PE
                # clock ramped through the load window
                warm = pp.tile([128, 2, 512], F8, name="warm")
                nc.gpsimd.memset(warm, 0.0)
                wwi = [0]

                def warm_touch(ch, n):
                    for _ in range(n):
                        wp = psp.tile([128, 512], F32, tag="pA", bufs=3,
                                      name=f"wt{wwi[0]}")
                        wwi[0] += 1
                        nc.tensor.matmul(
                            wp, ch[:, :, 0:128], warm, start=True, stop=True,
                            perf_mode=DR,
                        )

                for p in range(NPAIR):
                    warm_touch(kch[p], 4)
                for lb in range(NLB):
                    proj01(KT1, wk_sb, kch, lb, dve_cp)
                for p in range(NPAIR):
                    warm_touch(qch[p], 3)
                proj01(QT1, wq_sb, qch, 0, dve_cp)

            # ---- Stage B: attention + out-projection + RS + LN ----
            # 12 units (head, lblock); 2 units per phase, lb-major so each
            # l-block completes (and its Z+RS can launch) as early as possible.
            UNITS = [(lb, h) for lb in range(NLB) for h in range(HPC)]
            PHASES = [(UNITS[2 * i], UNITS[2 * i + 1]) for i in range(6)]

            def kq_slices(lb, h, ssl):
                lsl = slice(512 * lb, 512 * (lb + 1))
                if h == 0:
                    return KT1[0:32, :, ssl], QT1[0:32, :, lsl]
                if h == 1:
                    return KT1[32:64, :, ssl], QT1[32:64, :, lsl]
                return KT2[0:32, :, ssl], QT2[0:32, :, lsl]

            with (
                tc.tile_pool(name="ptp", bufs=1) as ptp,
                tc.tile_pool(name="drp", bufs=1) as drp,
                tc.tile_pool(name="zsb", bufs=4) as zsb,
                tc.tile_pool(name="aps", bufs=1, space="PSUM") as aps,
                tc.tile_pool(name="ep", bufs=2) as ep,
            ):
                def norm_drain(oU, lb, h, nm):
                    # oU [128, 512]: rows 0:64 = O*VS, row 64 = denom, 65: junk
                    lsl = slice(512 * lb, 512 * (lb + 1))
                    dn = drp.tile([1, 512], F32, tag="dn", bufs=3, name=f"dn{nm}")
                    nc.scalar.copy(out=dn, in_=oU[64:65, :])
                    rr = drp.tile([1, 512], F32, tag="rr", bufs=3, name=f"rr{nm}")
                    nc.vector.reciprocal_approx_fast(rr, dn)
                    rb = drp.tile([64, 512], F32, tag="rb", bufs=3, name=f"rb{nm}")
                    nc.gpsimd.partition_broadcast(rb, rr)
                    if h == 0:
                        nc.vector.tensor_mul(OT[0:64, 0, lsl], oU[0:64, :], rb)
                    elif h == 1:
                        nc.vector.tensor_mul(OT[64:96, 0, lsl], oU[0:32, :], rb[0:32])
                        nc.vector.tensor_mul(OT[0:32, 1, lsl], oU[32:64, :], rb[32:64])
                    else:
                        nc.vector.tensor_mul(OT[32:64, 1, lsl], oU[0:32, :], rb[0:32])
                        nc.vector.tensor_mul(OT[64:96, 1, lsl], oU[32:64, :], rb[32:64])

                def ln_chunk_ops(off, rows):
                    # LayerNorm over one received RS slot as deferrable closures
                    zr = ep.tile([128, D], F8, name="zr")[0:rows]
                    qr = ep.tile([128, D], BF16, name="qr")[0:rows]
                    xb = ep.tile([128, D], F32, name="xb")[0:rows]
                    stats = ep.tile([128, 3, 6], F32, name="st")[0:rows]
                    mv = ep.tile([128, 2], F32, name="mv")[0:rows]
                    ve = ep.tile([128, 1], F32, name="ve")[0:rows]
                    sh = ep.tile([128, 1], I32, name="sh")[0:rows]
                    r0i = ep.tile([128, 1], I32, name="r0i")[0:rows]
                    t2 = ep.tile([128, 1], F32, name="t2")[0:rows]
                    rstd = ep.tile([128, 1], F32, name="rs")[0:rows]
                    t1 = ep.tile([128, D], F32, name="t1")[0:rows]
                    o = ep.tile([128, D], F32, name="o")[0:rows]

                    def s_dma():
                        nc.sync.dma_start(out=zr, in_=Zr_dram[off : off + rows])
                        nc.sync.dma_start(out=qr, in_=qres[off : off + rows])

                    def s_add():
                        nc.gpsimd.tensor_scalar(
                            out=xb, in0=zr, scalar1=float(1.0 / ZS), scalar2=None,
                            op0=AL.mult,
                        )
                        nc.gpsimd.tensor_tensor(out=xb, in0=xb, in1=qr, op=AL.add)

                    def s_bn(g):
                        return lambda: nc.vector.bn_stats(
                            stats[:, g, :], xb[:, 256 * g : 256 * (g + 1)]
                        )

                    def s_aggr():
                        nc.vector.bn_aggr(mv, stats)
                        nc.vector.tensor_scalar_add(ve, mv[:, 1:2], float(LN_EPS))

                    def s_rsqrt():
                        nc.vector.tensor_scalar(
                            out=sh, in0=ve[:, 0:1].bitcast(I32), scalar1=1,
                            scalar2=None, op0=AL.arith_shift_right,
                        )
                        nc.vector.tensor_scalar(
                            out=r0i, in0=sh, scalar1=0x5F3759DF, scalar2=-1,
                            op0=AL.subtract, op1=AL.mult,
                        )
                        r0 = r0i[:, 0:1].bitcast(F32)
                        nc.vector.tensor_mul(t2, r0, r0)
                        nc.vector.tensor_mul(t2, t2, ve)
                        nc.vector.tensor_scalar(
                            out=t2, in0=t2, scalar1=-0.5, scalar2=1.5,
                            op0=AL.mult, op1=AL.add,
                        )
                        nc.vector.tensor_mul(rstd, r0, t2)

                    def s_stt1():
                        nc.gpsimd.tensor_tensor(
                            out=t1, in0=xb, in1=mv[:, 0:1].to_broadcast([rows, D]),
                            op=AL.subtract,
                        )
                        nc.gpsimd.tensor_tensor(
                            out=t1, in0=t1, in1=gam_b[0:rows], op=AL.mult
                        )

                    def s_stt2():
                        nc.vector.scalar_tensor_tensor(
                            o, t1, rstd, bet_b[0:rows], AL.mult, AL.add
                        )
                        nc.sync.dma_start(out=out[off : off + rows], in_=o)

                    return [s_dma, s_add, s_bn(0), s_bn(1), s_bn(2),
                            s_aggr, s_rsqrt, s_stt1, s_stt2]

                zi = [0]

                def z_tile(lb, t):
                    # Z = OT^T . wz (fp8 DR, contraction 192 = 2x96)
                    lt = 4 * lb + t
                    tsl = slice(128 * lt, 128 * (lt + 1))
                    zp = aps.tile([128, 1024], F32, tag="sp", bufs=1, name=f"zp{lt}")
                    nc.tensor.matmul(
                        zp[:, 0:512], OT[:, :, tsl], wz_sb[:, :, 0:512],
                        start=True, stop=True, perf_mode=DR,
                    )
                    nc.tensor.matmul(
                        zp[:, 512:768], OT[:, :, tsl], wz_sb[:, :, 512:768],
                        start=True, stop=True, perf_mode=DR,
                    )
                    zs = zsb.tile([128, D], F8, name="zs")
                    if zi[0] % 2 == 0:
                        nc.scalar.copy(out=zs, in_=zp[:, 0:768])
                    else:
                        nc.vector.tensor_copy(out=zs, in_=zp[:, 0:768])
                    zi[0] += 1
                    nc.sync.dma_start(out=Z_dram[tsl, :], in_=zs)
                    if lb < 3 and t == 3:
                        nc.gpsimd.collective_compute(
                            "ReduceScatter",
                            AL.add,
                            replica_groups=GROUPS,
                            ins=[Z_dram[512 * lb : 512 * (lb + 1), :].opt()],
                            outs=[Zr_dram[128 * lb : 128 * (lb + 1)].opt()],
                        )
                        pend_rs.add(lb)
                    elif lb == 3 and t == 3:
                        nc.gpsimd.collective_compute(
                            "ReduceScatter",
                            AL.add,
                            replica_groups=GROUPS,
                            ins=[Z_dram[1536:2048, :].opt()],
                            outs=[Zr_dram[384:512].opt()],
                        )
                        pend_rs.add(3)

                def projB01(dst, w_sb, src_, lb):
                    def go():
                        lsl = slice(512 * lb, 512 * (lb + 1))
                        ps = aps.tile([128, 1024], F32, tag="sp", bufs=1, name="pj")
                        for p in range(NPAIR):
                            nc.tensor.matmul(
                                ps[:, 0:512], w_sb[:, p, :, 0:128], src_[p][:, :, lsl],
                                start=(p == 0), stop=(p == NPAIR - 1), perf_mode=DR,
                            )
                        for t in range(2):
                            nc.vector.tensor_copy(
                                out=dst[:, t, lsl], in_=ps[64 * t : 64 * (t + 1), 0:512]
                            )
                    return go

                def projB2(dst, w_sb, src_, lb):
                    def go():
                        lsl = slice(512 * lb, 512 * (lb + 1))
                        ps = aps.tile([128, 1024], F32, tag="sp", bufs=1, name="pj2")
                        for p in range(NPAIR):
                            nc.tensor.matmul(
                                ps[0:64, 0:512], w_sb[:, p, :, 128:192],
                                src_[p][:, :, lsl],
                                start=(p == 0), stop=(p == NPAIR - 1), perf_mode=DR,
                            )
                        for t in range(2):
                            nc.scalar.copy(
                                out=dst[:, t, lsl], in_=ps[32 * t : 32 * (t + 1), 0:512]
                            )
                    return go

                def v_proj(sc):
                    def go():
                        ssl = slice(128 * sc, 128 * (sc + 1))
                        pv = aps.tile([128, 1024], F32, tag="sp", bufs=1, name="pv")
                        for p in range(NPAIR):
                            nc.tensor.matmul(
                                pv[:, 0:HCOLS], vch[p][:, :, ssl], wv_sb[:, p, :, :],
                                start=(p == 0), stop=(p == NPAIR - 1), perf_mode=DR,
                            )
                        nc.scalar.copy(
                            out=V_sb[:, sc // 2, :, sc % 2, 0:64],
                            in_=pv[:, 0:HCOLS].rearrange("p (h d) -> p h d", h=HPC),
                        )
                    return go

                pend_v = [v_proj(sc) for sc in range(SCH)]

                # remaining projections spread through early phases;
                # NEED[ph] = how many must have been emitted by end of phase ph
                pend_proj = [projB2(KT2, wk_sb, kch, lb) for lb in range(NLB)]
                pend_proj += [
                    projB2(QT2, wq_sb, qch, 0),
                    projB01(QT1, wq_sb, qch, 1),
                    projB2(QT2, wq_sb, qch, 1),
                    projB01(QT1, wq_sb, qch, 2),
                    projB2(QT2, wq_sb, qch, 2),
                    projB01(QT1, wq_sb, qch, 3),
                    projB2(QT2, wq_sb, qch, 3),
                ]
                proj_done = [0]
                PROJ_NEED = {0: 6, 1: 7, 2: 8, 3: 10, 4: 11, 5: 11}

                pend_rs = set()
                ln_queued = set()
                bg_ops = []
                pend_z = []
                ln_wait = []
                ln_wait2 = []
                done_units = set()

                def queue_ready_lns():
                    # defer one phase after the RS fires so in-order engine
                    # queues never head-of-line block on collective latency
                    for s in range(4):
                        if s not in ln_queued and s in pend_rs:
                            ln_queued.add(s)
                            ln_wait.append(s)

                def attn_phase(ph):
                    (lbA, hA), (lbB, hB) = PHASES[ph]
                    oA = aps.tile([128, 512], F32, tag="oA", bufs=1, name=f"oA{ph}")
                    oB = aps.tile([128, 512], F32, tag="oB", bufs=1, name=f"oB{ph}")
                    P = None
                    Ps = []

                    def emit_av(p):
                        nc.tensor.matmul(
                            oA, V_sb[:, p, hA, :, :], Ps[p][:, 0, :, :],
                            start=(p == 0), stop=(p == 7), perf_mode=DR,
                        )
                        nc.tensor.matmul(
                            oB, V_sb[:, p, hB, :, :], Ps[p][:, 1, :, :],
                            start=(p == 0), stop=(p == 7), perf_mode=DR,
                        )
                    for sc in range(SCH):
                        pair, par = sc // 2, sc % 2
                        ssl = slice(128 * sc, 128 * (sc + 1))
                        sw = aps.tile(
                            [128, 1024], F32, tag="sw", bufs=2, name=f"sw{ph}_{sc}"
                        )
                        kA, qA = kq_slices(lbA, hA, ssl)
                        kB, qB = kq_slices(lbB, hB, ssl)
                        nc.tensor.matmul(
                            sw[:, 0:512], kA, qA, start=True, stop=True, perf_mode=DR
                        )
                        nc.tensor.matmul(
                            sw[:, 512:1024], kB, qB, start=True, stop=True, perf_mode=DR
                        )
                        if par == 0:
                            P = ptp.tile(
                                [128, 2, 2, 512], F8, tag="p", bufs=4, name=f"P{ph}_{pair}"
                            )
                            Ps.append(P)
                        pdst = P[:, :, par, :]
                        if EXP_SCALAR[sc]:
                            nc.scalar.activation(
                                pdst, sw, ACT.Exp,
                                bias=asc_bias[:, sc : sc + 1], scale=float(1.0 / A8),
                            )
                        else:
                            nc.vector.tensor_scalar(
                                out=pdst.bitcast(I8), in0=sw,
                                scalar1=dve_bias[:, sc : sc + 1], scalar2=0.0,
                                op0=AL.add, op1=AL.max,
                            )
                        if par == 1 and pair >= 2:
                            emit_av(pair - 2)
                        if pend_v:
                            pend_v.pop(0)()
                            if pend_v:
                                pend_v.pop(0)()
                        elif par == 0 and pend_proj:
                            pend_proj.pop(0)()
                            proj_done[0] += 1
                        if par == 1 and pend_z:
                            pend_z.pop(0)()
                        if bg_ops:
                            bg_ops.pop(0)()
                    emit_av(6)
                    emit_av(7)
                    while pend_z:
                        pend_z.pop(0)()
                    while proj_done[0] < PROJ_NEED[ph] and pend_proj:
                        pend_proj.pop(0)()
                        proj_done[0] += 1
                    norm_drain(oA, lbA, hA, f"{ph}a")
                    norm_drain(oB, lbB, hB, f"{ph}b")
                    done_units.update(((lbA, hA), (lbB, hB)))
                    for lb in range(NLB):
                        if (lb, 2) in done_units and (lb, 0) in done_units and \
                           (lb, 1) in done_units and lb not in z_done:
                            z_done.add(lb)
                            for t_ in range(4):
                                pend_z.append(
                                    (lambda lb_=lb, t__=t_: z_tile(lb_, t__))
                                )
                    queue_ready_lns()
                    while ln_wait2:
                        bg_ops.extend(ln_chunk_ops(128 * ln_wait2.pop(0), 128))
                    ln_wait2.extend(ln_wait)
                    del ln_wait[:]

                z_done = set()
                for ph in range(6):
                    attn_phase(ph)
                while pend_z:
                    pend_z.pop(0)()
                while bg_ops:
                    bg_ops.pop(0)()
                queue_ready_lns()
                for s in ln_wait2 + ln_wait:
                    bg_ops.extend(ln_chunk_ops(128 * s, 128))
                del ln_wait2[:], ln_wait[:]
                while bg_ops:
                    bg_ops.pop(0)()

    nc.finalize()
    return nc


def _get_nc():
    if "nc" not in _CACHE:
        _CACHE["nc"] = _build()
    return _CACHE["nc"]


def build_in_maps(inputs):
    return _build_in_maps(**inputs)


F8NP = ml_dtypes.float8_e4m3fn


def _f8(x):
    return np.ascontiguousarray(np.asarray(x, dtype=np.float32).astype(F8NP))


def _bf(x):
    return np.ascontiguousarray(np.asarray(x, dtype=np.float32).astype(ml_dtypes.bfloat16))


def _pack_in(xT):
    # [D, N] -> [3, 128, 2, N] DR pairs
    return np.ascontiguousarray(
        xT.reshape(NPAIR, 2, 128, xT.shape[1]).transpose(0, 2, 1, 3)
    )


def _pack_w(wT):
    # [D, 192] -> [128, 3, 2, 192] DR pairs
    return np.ascontiguousarray(
        wT.reshape(NPAIR, 2, 128, HCOLS).transpose(2, 0, 1, 3)
    )


# psum partition order for the h0/h1 projection block + h2 block
_PERM = np.r_[0:32, 64:96, 32:64, 96:128, 128:160, 160:192]


def _build_in_maps(q, k, v, attention_mask, Wq, Wk, Wv, W, b, gamma, beta):
    q = np.asarray(q, dtype=np.float32)
    k = np.asarray(k, dtype=np.float32)
    v = np.asarray(v, dtype=np.float32)
    attention_mask = np.asarray(attention_mask, dtype=np.int32)
    Wq = np.asarray(Wq, dtype=np.float32)
    Wk = np.asarray(Wk, dtype=np.float32)
    Wv = np.asarray(Wv, dtype=np.float32)
    W = np.asarray(W, dtype=np.float32)
    b = np.asarray(b, dtype=np.float32)
    gamma = np.asarray(gamma, dtype=np.float32)
    beta = np.asarray(beta, dtype=np.float32)

    qT = [_pack_in(_f8(q[i].T)) for i in range(B)]
    kT = [_pack_in(_f8(k[i].T)) for i in range(B)]
    vT = [_pack_in(_f8(v[i].T)) for i in range(B)]
    maskT = [
        np.ascontiguousarray(attention_mask[i].reshape(SCH, 128).T) for i in range(B)
    ]
    gamma1 = np.ascontiguousarray(gamma.reshape(1, D))
    beta1 = np.ascontiguousarray(beta.reshape(1, D))

    in_maps = []
    for c in range(NCORES):
        bi, hg = c // 4, c % 4
        cs = slice(HCOLS * hg, HCOLS * (hg + 1))
        wqT = Wq[cs, :][_PERM].T * SQ  # [768, 192]
        wkT = Wk[cs, :][_PERM].T
        wvT = Wv[cs, :].T * VS
        wzT = W[:, cs].T * WS  # [192, 768]
        in_maps.append(
            {
                "qT8": qT[bi],
                "kT8": kT[bi],
                "vT8": vT[bi],
                "wq8": _pack_w(_f8(wqT)),
                "wk8": _pack_w(_f8(wkT)),
                "wv8": _pack_w(_f8(wvT)),
                "wz8": np.ascontiguousarray(
                    _f8(wzT).reshape(2, 96, D).transpose(1, 0, 2)
                ),
                "qres": _bf(
                    np.concatenate(
                        [q[bi, 512 * j + 128 * hg : 512 * j + 128 * (hg + 1), :]
                         for j in range(4)]
                    )
                    + b[None, :]
                ),
                "maskT": maskT[bi],
                "gamma1": gamma1,
                "beta1": beta1,
            }
        )
    return in_maps


def kernel(q, k, v, attention_mask, Wq, Wk, Wv, W, b, gamma, beta):
    nc = _get_nc()
    in_maps = _build_in_maps(q, k, v, attention_mask, Wq, Wk, Wv, W, b, gamma, beta)
    res = run_bass_kernel_spmd(nc, in_maps, core_ids=list(range(NCORES)))

    outp = np.empty((B, L, D), dtype=np.float32)
    for c in range(NCORES):
        bi, hg = c // 4, c % 4
        o = res.results[c]["out"]
        for j in range(4):
            outp[bi, 512 * j + 128 * hg : 512 * j + 128 * (hg + 1), :] = o[128 * j : 128 * (j + 1)]
    return outp


# revision 15
# speedup vs baseline: 1.4317x; 1.4317x over previous
"""Trainium2 Bass kernel for fused MultiHeadAttention + residual + LayerNorm.

Problem: B=2, L=S=2048, D=768, H=12 heads of dim 64, attention with key-padding
mask, output projection + bias, residual with q, LayerNorm(gamma, beta).

Sharding over 8 NeuronCores: data-parallel over batch (2 groups of 4 cores) x
tensor-parallel over heads (3 heads per core).

V3: full fp8e4m3 DoubleRow pipeline. All matmuls (QKV projections, QK^T
scores, attention-value, output projection) run in fp8 DoubleRow perf mode
(0.5 cycles/row, two 128-row k-tiles per pass = 4x fewer PE cycles vs bf16).
Q is pre-scaled by 1/ln2 in the wq weights so score PSUM arrives as
A8*(s/8); softmax exp splits between ScalarE (table exp, f8 out) and DVE
(one-shot Schraudolph writing e4m3 bits as int8). Denominator via a ones
column at V slot 64 -> PSUM partition 64; norm chain: ScalarE denom copy ->
DVE reciprocal_approx_fast -> Pool partition_broadcast -> DVE multiply.
V scaled x32 and W x8 so the f8 O/Z stores stay in normal range (Z carries
x256, removed in the LayerNorm residual add). ReduceScatter in 8 fp8 chunks;
LayerNorm on chunk pairs (Pool handles the elementwise stages, DVE the
bn_stats/rsqrt) with the last two chunks split for a short tail.
"""

import sys

sys.path.insert(0, "/opt/trn_rl_repo")

import ml_dtypes
import numpy as np

import concourse.bass as bass
import concourse.tile as tile
from concourse import bacc, mybir
from concourse.bass_utils import run_bass_kernel_spmd

F32 = mybir.dt.float32
BF16 = mybir.dt.bfloat16
F8 = mybir.dt.float8e4
I8 = mybir.dt.int8
I32 = mybir.dt.int32
DR = mybir.MatmulPerfMode.DoubleRow

D = 768
HD = 64
HPC = 3  # heads per core
HCOLS = HPC * HD  # 192
B, L, S = 2, 2048, 2048
NCORES = 8
GROUPS = [[0, 1, 2, 3], [4, 5, 6, 7]]
NPAIR = 3  # d-chunk DR pairs for projections (768 = 3 * 256)
SCH = S // 128  # 16 s-chunks
LB = 512
NLB = L // LB  # 4
NCHUNK = 8  # output chunk slots (64 rows/core); RS merged per l-block, split for the last
CROWS = L // NCHUNK  # 256
ORows = CROWS // 4  # 64 rows per core per chunk
LN_EPS = 1e-5
MASK_NEG = -1000000.0

# fp8 Schraudolph / scaling constants
A8 = 8.0 / np.log(2.0)  # e4m3 bits per e-fold
B8C = 7 * 8 - 0.46  # e4m3 exponent bias bits - Schraudolph C
M_SHIFT = 3.0  # global logit shift (softmax-invariant), keeps bits < 127
SQ = 1.0 / np.log(2.0)  # = A8/8, folded into wq
VS = 32.0  # V scale (keeps f8 O in normal range)
WS = 8.0  # W scale (keeps f8 W in normal range)
ZS = VS * WS  # Z carries x256

# exp engine split: True -> ScalarE table exp, False -> DVE Schraudolph
EXP_SCALAR = (True, True, True, False, True, False, True, False,
              True, False, True, False, True, False, True, False)  # 9 S / 7 D

_CACHE: dict = {}


def _build():
    nc = bacc.Bacc("TRN2", target_bir_lowering=False, debug=False, num_devices=NCORES)

    qT8 = nc.dram_tensor("qT8", [NPAIR, 128, 2, L], F8, kind="ExternalInput").ap()
    kT8 = nc.dram_tensor("kT8", [NPAIR, 128, 2, S], F8, kind="ExternalInput").ap()
    vT8 = nc.dram_tensor("vT8", [NPAIR, 128, 2, S], F8, kind="ExternalInput").ap()
    wq8 = nc.dram_tensor("wq8", [128, NPAIR, 2, HCOLS], F8, kind="ExternalInput").ap()
    wk8 = nc.dram_tensor("wk8", [128, NPAIR, 2, HCOLS], F8, kind="ExternalInput").ap()
    wv8 = nc.dram_tensor("wv8", [128, NPAIR, 2, HCOLS], F8, kind="ExternalInput").ap()
    wz8 = nc.dram_tensor("wz8", [96, 2, D], F8, kind="ExternalInput").ap()
    qres = nc.dram_tensor("qres", [512, D], BF16, kind="ExternalInput").ap()
    maskT = nc.dram_tensor("maskT", [128, SCH], I32, kind="ExternalInput").ap()
    gamma1 = nc.dram_tensor("gamma1", [1, D], F32, kind="ExternalInput").ap()
    beta1 = nc.dram_tensor("beta1", [1, D], F32, kind="ExternalInput").ap()
    out = nc.dram_tensor("out", [512, D], F32, kind="ExternalOutput").ap()

    AL = mybir.AluOpType
    ACT = mybir.ActivationFunctionType

    with tile.TileContext(nc, num_cores=NCORES) as tc:
        with (
            tc.tile_pool(name="persist", bufs=1) as pp,
            tc.tile_pool(name="dram", bufs=1, space="DRAM") as dram,
        ):
            kch = [pp.tile([128, 2, S], F8, name=f"kch{p}") for p in range(NPAIR)]
            vch = [pp.tile([128, 2, S], F8, name=f"vch{p}") for p in range(NPAIR)]
            qch = [pp.tile([128, 2, L], F8, name=f"qch{p}") for p in range(NPAIR)]
            wq_sb = pp.tile([128, NPAIR, 2, HCOLS], F8)
            wk_sb = pp.tile([128, NPAIR, 2, HCOLS], F8)
            wv_sb = pp.tile([128, NPAIR, 2, HCOLS], F8)
            wz_sb = pp.tile([96, 2, D], F8)
            KT1 = pp.tile([64, 2, S], F8)  # h0 on 0:32, h1 on 32:64; slot=lo/hi
            QT1 = pp.tile([64, 2, L], F8)
            KT2 = pp.tile([32, 2, S], F8)  # h2
            QT2 = pp.tile([32, 2, L], F8)
            V_sb = pp.tile([128, SCH // 2, HPC, 2, 128], F8)  # col 64 = ones, 65: pad
            OT = pp.tile([96, 2, L], F8)  # z contraction layout
            mask_i = pp.tile([128, SCH], I32)
            mask_f = pp.tile([128, SCH], F32)
            asc_bias = pp.tile([128, SCH], F32)
            dve_bias = pp.tile([128, SCH], F32)
            gam_b = pp.tile([128, D], F32)
            bet_b = pp.tile([128, D], F32)

            Z_dram = dram.tile([L, D], F8)
            Zr_dram = dram.tile([512, D], F8)
            sync_in = dram.tile([4, 192], BF16)
            sync_out = dram.tile([1, 192], BF16)

            # early dummy collective: absorbs inter-core rendezvous skew
            nc.gpsimd.collective_compute(
                "ReduceScatter",
                AL.add,
                replica_groups=GROUPS,
                ins=[sync_in[:, :].opt()],
                outs=[sync_out[:, :].opt()],
            )
            # weight / constant loads
            nc.sync.dma_start(out=wq_sb, in_=wq8[:, :, :, :])
            nc.sync.dma_start(out=wk_sb, in_=wk8[:, :, :, :])
            nc.sync.dma_start(out=wv_sb, in_=wv8[:, :, :, :])
            nc.sync.dma_start(out=wz_sb, in_=wz8[:, :, :])
            nc.sync.dma_start(out=mask_i, in_=maskT[:, :])
            nc.sync.dma_start(out=gam_b, in_=gamma1.to_broadcast([128, D]))
            nc.sync.dma_start(out=bet_b, in_=beta1.to_broadcast([128, D]))
            # input loads: k first (scores need full K), then q, then v
            for p in range(NPAIR):
                nc.sync.dma_start(out=kch[p], in_=kT8[p])
            for p in range(NPAIR):
                nc.sync.dma_start(out=qch[p], in_=qT8[p])
            for p in range(NPAIR):
                nc.sync.dma_start(out=vch[p], in_=vT8[p])

            nc.vector.tensor_copy(mask_f, mask_i)
            # ScalarE exp bias: (1-m)*MASK_NEG - M
            nc.scalar.activation(
                asc_bias, mask_f, ACT.Copy,
                bias=float(MASK_NEG - M_SHIFT), scale=-MASK_NEG,
            )
            # DVE Schraudolph bias: A8*((1-m)*MASK_NEG - M) + B8C
            nc.scalar.activation(
                dve_bias, mask_f, ACT.Copy,
                bias=float(A8 * (MASK_NEG - M_SHIFT) + B8C), scale=float(-A8 * MASK_NEG),
            )
            # ones column for softmax denominators (pad cols zeroed)
            nc.gpsimd.memset(V_sb[:, :, :, :, 64:128], 0.0)
            ones_t = pp.tile([128, SCH // 2, HPC, 2, 1], F8)
            nc.gpsimd.memset(ones_t, 1.0)
            nc.vector.tensor_copy(V_sb[:, :, :, :, 64:65], ones_t)

            # ---- Stage A: projections (fp8 DoubleRow) ----
            with tc.tile_pool(name="psp", bufs=1, space="PSUM") as psp:
                def proj01(dst, w_sb, src, lb, drain_eng):
                    lsl = slice(512 * lb, 512 * (lb + 1))
                    ps = psp.tile([128, 512], F32, tag="pA", bufs=3, name="p01")
                    for p in range(NPAIR):
                        nc.tensor.matmul(
                            ps, w_sb[:, p, :, 0:128], src[p][:, :, lsl],
                            start=(p == 0), stop=(p == NPAIR - 1), perf_mode=DR,
                        )
                    for t in range(2):
                        drain_eng(out=dst[:, t, lsl], in_=ps[64 * t : 64 * (t + 1)])

                def proj2(dst, w_sb, src, lb, drain_eng):
                    lsl = slice(512 * lb, 512 * (lb + 1))
                    ps = psp.tile([64, 512], F32, tag="pB", bufs=2, name="p2")
                    for p in range(NPAIR):
                        nc.tensor.matmul(
                            ps, w_sb[:, p, :, 128:192], src[p][:, :, lsl],
                            start=(p == 0), stop=(p == NPAIR - 1), perf_mode=DR,
                        )
                    for t in range(2):
                        drain_eng(out=dst[:, t, lsl], in_=ps[32 * t : 32 * (t + 1)])

                dve_cp = nc.vector.tensor_copy
                sc_cp = nc.scalar.copy

                # warm fillers keyed on the input DMA pairs keep the PE
                # clock ramped through the load window
                warm = pp.tile([128, 2, 512], F8, name="warm")
                nc.gpsimd.memset(warm, 0.0)
                wwi = [0]

                def warm_touch(ch, n):
                    for _ in range(n):
                        wp = psp.tile([128, 512], F32, tag="pA", bufs=3,
                                      name=f"wt{wwi[0]}")
                        wwi[0] += 1
                        nc.tensor.matmul(
                            wp, ch[:, :, 0:128], warm, start=True, stop=True,
                            perf_mode=DR,
                        )

                for p in range(NPAIR):
                    warm_touch(kch[p], 4)
                for lb in range(NLB):
                    proj01(KT1, wk_sb, kch, lb, dve_cp)
                for p in range(NPAIR):
                    warm_touch(qch[p], 3)
                proj01(QT1, wq_sb, qch, 0, dve_cp)

            # ---- Stage B: attention + out-projection + RS + LN ----
            # 12 units (head, lblock); 2 units per phase, lb-major so each
            # l-block completes (and its Z+RS can launch) as early as possible.
            UNITS = [(lb, h) for lb in range(NLB) for h in range(HPC)]
            PHASES = [(UNITS[2 * i], UNITS[2 * i + 1]) for i in range(6)]

            def kq_slices(lb, h, ssl):
                lsl = slice(512 * lb, 512 * (lb + 1))
                if h == 0:
                    return KT1[0:32, :, ssl], QT1[0:32, :, lsl]
                if h == 1:
                    return KT1[32:64, :, ssl], QT1[32:64, :, lsl]
                return KT2[0:32, :, ssl], QT2[0:32, :, lsl]

            with (
                tc.tile_pool(name="ptp", bufs=1) as ptp,
                tc.tile_pool(name="drp", bufs=1) as drp,
                tc.tile_pool(name="zsb", bufs=4) as zsb,
                tc.tile_pool(name="aps", bufs=1, space="PSUM") as aps,
                tc.tile_pool(name="ep", bufs=2) as ep,
            ):
                def norm_drain(oU, lb, h, nm):
                    # oU [128, 512]: rows 0:64 = O*VS, row 64 = denom, 65: junk
                    lsl = slice(512 * lb, 512 * (lb + 1))
                    dn = drp.tile([1, 512], F32, tag="dn", bufs=3, name=f"dn{nm}")
                    nc.scalar.copy(out=dn, in_=oU[64:65, :])
                    rr = drp.tile([1, 512], F32, tag="rr", bufs=3, name=f"rr{nm}")
                    nc.vector.reciprocal_approx_fast(rr, dn)
                    rb = drp.tile([64, 512], F32, tag="rb", bufs=3, name=f"rb{nm}")
                    nc.gpsimd.partition_broadcast(rb, rr)
                    if h == 0:
                        nc.vector.tensor_mul(OT[0:64, 0, lsl], oU[0:64, :], rb)
                    elif h == 1:
                        nc.vector.tensor_mul(OT[64:96, 0, lsl], oU[0:32, :], rb[0:32])
                        nc.vector.tensor_mul(OT[0:32, 1, lsl], oU[32:64, :], rb[32:64])
                    else:
                        nc.vector.tensor_mul(OT[32:64, 1, lsl], oU[0:32, :], rb[0:32])
                        nc.vector.tensor_mul(OT[64:96, 1, lsl], oU[32:64, :], rb[32:64])

                def ln_chunk_ops(off, rows):
                    # LayerNorm over one received RS slot as deferrable closures
                    zr = ep.tile([128, D], F8, name="zr")[0:rows]
                    qr = ep.tile([128, D], BF16, name="qr")[0:rows]
                    xb = ep.tile([128, D], F32, name="xb")[0:rows]
                    stats = ep.tile([128, 3, 6], F32, name="st")[0:rows]
                    mv = ep.tile([128, 2], F32, name="mv")[0:rows]
                    ve = ep.tile([128, 1], F32, name="ve")[0:rows]
                    sh = ep.tile([128, 1], I32, name="sh")[0:rows]
                    r0i = ep.tile([128, 1], I32, name="r0i")[0:rows]
                    t2 = ep.tile([128, 1], F32, name="t2")[0:rows]
                    rstd = ep.tile([128, 1], F32, name="rs")[0:rows]
                    t1 = ep.tile([128, D], F32, name="t1")[0:rows]
                    o = ep.tile([128, D], F32, name="o")[0:rows]

                    def s_dma():
                        nc.sync.dma_start(out=zr, in_=Zr_dram[off : off + rows])
                        nc.sync.dma_start(out=qr, in_=qres[off : off + rows])

                    def s_add():
                        nc.vector.scalar_tensor_tensor(
                            xb, zr, float(1.0 / ZS), qr, AL.mult, AL.add
                        )

                    def s_bn(g):
                        return lambda: nc.vector.bn_stats(
                            stats[:, g, :], xb[:, 256 * g : 256 * (g + 1)]
                        )

                    def s_aggr():
                        nc.vector.bn_aggr(mv, stats)
                        nc.vector.tensor_scalar_add(ve, mv[:, 1:2], float(LN_EPS))

                    def s_rsqrt():
                        nc.vector.tensor_scalar(
                            out=sh, in0=ve[:, 0:1].bitcast(I32), scalar1=1,
                            scalar2=None, op0=AL.arith_shift_right,
                        )
                        nc.vector.tensor_scalar(
                            out=r0i, in0=sh, scalar1=0x5F3759DF, scalar2=-1,
                            op0=AL.subtract, op1=AL.mult,
                        )
                        r0 = r0i[:, 0:1].bitcast(F32)
                        nc.vector.tensor_mul(t2, r0, r0)
                        nc.vector.tensor_mul(t2, t2, ve)
                        nc.vector.tensor_scalar(
                            out=t2, in0=t2, scalar1=-0.5, scalar2=1.5,
                            op0=AL.mult, op1=AL.add,
                        )
                        nc.vector.tensor_mul(rstd, r0, t2)

                    def s_stt1():
                        nc.vector.scalar_tensor_tensor(
                            t1, xb, mv[:, 0:1], gam_b[0:rows], AL.subtract, AL.mult
                        )

                    def s_stt2():
                        nc.vector.scalar_tensor_tensor(
                            o, t1, rstd, bet_b[0:rows], AL.mult, AL.add
                        )
                        nc.sync.dma_start(out=out[off : off + rows], in_=o)

                    return [s_dma, s_add, s_bn(0), s_bn(1), s_bn(2),
                            s_aggr, s_rsqrt, s_stt1, s_stt2]

                zi = [0]

                def z_tile(lb, t):
                    # Z = OT^T . wz (fp8 DR, contraction 192 = 2x96)
                    lt = 4 * lb + t
                    tsl = slice(128 * lt, 128 * (lt + 1))
                    zp = aps.tile([128, 1024], F32, tag="sp", bufs=1, name=f"zp{lt}")
                    nc.tensor.matmul(
                        zp[:, 0:512], OT[:, :, tsl], wz_sb[:, :, 0:512],
                        start=True, stop=True, perf_mode=DR,
                    )
                    nc.tensor.matmul(
                        zp[:, 512:768], OT[:, :, tsl], wz_sb[:, :, 512:768],
                        start=True, stop=True, perf_mode=DR,
                    )
                    zs = zsb.tile([128, D], F8, name="zs")
                    if zi[0] % 2 == 0:
                        nc.scalar.copy(out=zs, in_=zp[:, 0:768])
                    else:
                        nc.vector.tensor_copy(out=zs, in_=zp[:, 0:768])
                    zi[0] += 1
                    nc.sync.dma_start(out=Z_dram[tsl, :], in_=zs)
                    if lb < 3 and t == 3:
                        nc.gpsimd.collective_compute(
                            "ReduceScatter",
                            AL.add,
                            replica_groups=GROUPS,
                            ins=[Z_dram[512 * lb : 512 * (lb + 1), :].opt()],
                            outs=[Zr_dram[128 * lb : 128 * (lb + 1)].opt()],
                        )
                        pend_rs.add(lb)
                    elif lb == 3 and t == 3:
                        nc.gpsimd.collective_compute(
                            "ReduceScatter",
                            AL.add,
                            replica_groups=GROUPS,
                            ins=[Z_dram[1536:2048, :].opt()],
                            outs=[Zr_dram[384:512].opt()],
                        )
                        pend_rs.add(3)

                def projB01(dst, w_sb, src_, lb):
                    def go():
                        lsl = slice(512 * lb, 512 * (lb + 1))
                        ps = aps.tile([128, 1024], F32, tag="sp", bufs=1, name="pj")
                        for p in range(NPAIR):
                            nc.tensor.matmul(
                                ps[:, 0:512], w_sb[:, p, :, 0:128], src_[p][:, :, lsl],
                                start=(p == 0), stop=(p == NPAIR - 1), perf_mode=DR,
                            )
                        for t in range(2):
                            nc.vector.tensor_copy(
                                out=dst[:, t, lsl], in_=ps[64 * t : 64 * (t + 1), 0:512]
                            )
                    return go

                def projB2(dst, w_sb, src_, lb):
                    def go():
                        lsl = slice(512 * lb, 512 * (lb + 1))
                        ps = aps.tile([128, 1024], F32, tag="sp", bufs=1, name="pj2")
                        for p in range(NPAIR):
                            nc.tensor.matmul(
                                ps[0:64, 0:512], w_sb[:, p, :, 128:192],
                                src_[p][:, :, lsl],
                                start=(p == 0), stop=(p == NPAIR - 1), perf_mode=DR,
                            )
                        for t in range(2):
                            nc.scalar.copy(
                                out=dst[:, t, lsl], in_=ps[32 * t : 32 * (t + 1), 0:512]
                            )
                    return go

                def v_proj(sc):
                    def go():
                        ssl = slice(128 * sc, 128 * (sc + 1))
                        pv = aps.tile([128, 1024], F32, tag="sp", bufs=1, name="pv")
                        for p in range(NPAIR):
                            nc.tensor.matmul(
                                pv[:, 0:HCOLS], vch[p][:, :, ssl], wv_sb[:, p, :, :],
                                start=(p == 0), stop=(p == NPAIR - 1), perf_mode=DR,
                            )
                        nc.scalar.copy(
                            out=V_sb[:, sc // 2, :, sc % 2, 0:64],
                            in_=pv[:, 0:HCOLS].rearrange("p (h d) -> p h d", h=HPC),
                        )
                    return go

                pend_v = [v_proj(sc) for sc in range(SCH)]

                # remaining projections spread through early phases;
                # NEED[ph] = how many must have been emitted by end of phase ph
                pend_proj = [projB2(KT2, wk_sb, kch, lb) for lb in range(NLB)]
                pend_proj += [
                    projB2(QT2, wq_sb, qch, 0),
                    projB01(QT1, wq_sb, qch, 1),
                    projB2(QT2, wq_sb, qch, 1),
                    projB01(QT1, wq_sb, qch, 2),
                    projB2(QT2, wq_sb, qch, 2),
                    projB01(QT1, wq_sb, qch, 3),
                    projB2(QT2, wq_sb, qch, 3),
                ]
                proj_done = [0]
                PROJ_NEED = {0: 6, 1: 7, 2: 8, 3: 10, 4: 11, 5: 11}

                pend_rs = set()
                ln_queued = set()
                bg_ops = []
                pend_z = []
                ln_wait = []
                ln_wait2 = []
                done_units = set()

                def queue_ready_lns():
                    # defer one phase after the RS fires so in-order engine
                    # queues never head-of-line block on collective latency
                    for s in range(4):
                        if s not in ln_queued and s in pend_rs:
                            ln_queued.add(s)
                            ln_wait.append(s)

                def attn_phase(ph):
                    (lbA, hA), (lbB, hB) = PHASES[ph]
                    oA = aps.tile([128, 512], F32, tag="oA", bufs=1, name=f"oA{ph}")
                    oB = aps.tile([128, 512], F32, tag="oB", bufs=1, name=f"oB{ph}")
                    P = None
                    Ps = []

                    def emit_av(p):
                        nc.tensor.matmul(
                            oA, V_sb[:, p, hA, :, :], Ps[p][:, 0, :, :],
                            start=(p == 0), stop=(p == 7), perf_mode=DR,
                        )
                        nc.tensor.matmul(
                            oB, V_sb[:, p, hB, :, :], Ps[p][:, 1, :, :],
                            start=(p == 0), stop=(p == 7), perf_mode=DR,
                        )
                    for sc in range(SCH):
                        pair, par = sc // 2, sc % 2
                        ssl = slice(128 * sc, 128 * (sc + 1))
                        sw = aps.tile(
                            [128, 1024], F32, tag="sw", bufs=2, name=f"sw{ph}_{sc}"
                        )
                        kA, qA = kq_slices(lbA, hA, ssl)
                        kB, qB = kq_slices(lbB, hB, ssl)
                        nc.tensor.matmul(
                            sw[:, 0:512], kA, qA, start=True, stop=True, perf_mode=DR
                        )
                        nc.tensor.matmul(
                            sw[:, 512:1024], kB, qB, start=True, stop=True, perf_mode=DR
                        )
                        if par == 0:
                            P = ptp.tile(
                                [128, 2, 2, 512], F8, tag="p", bufs=4, name=f"P{ph}_{pair}"
                            )
                            Ps.append(P)
                        pdst = P[:, :, par, :]
                        if EXP_SCALAR[sc]:
                            nc.scalar.activation(
                                pdst, sw, ACT.Exp,
                                bias=asc_bias[:, sc : sc + 1], scale=float(1.0 / A8),
                            )
                        else:
                            nc.vector.tensor_scalar(
                                out=pdst.bitcast(I8), in0=sw,
                                scalar1=dve_bias[:, sc : sc + 1], scalar2=0.0,
                                op0=AL.add, op1=AL.max,
                            )
                        if par == 1 and pair >= 2:
                            emit_av(pair - 2)
                        if pend_v:
                            pend_v.pop(0)()
                            if pend_v:
                                pend_v.pop(0)()
                        elif par == 0 and pend_proj:
                            pend_proj.pop(0)()
                            proj_done[0] += 1
                        if par == 1 and pend_z:
                            pend_z.pop(0)()
                        if bg_ops:
                            bg_ops.pop(0)()
                    emit_av(6)
                    emit_av(7)
                    while pend_z:
                        pend_z.pop(0)()
                    while proj_done[0] < PROJ_NEED[ph] and pend_proj:
                        pend_proj.pop(0)()
                        proj_done[0] += 1
                    norm_drain(oA, lbA, hA, f"{ph}a")
                    norm_drain(oB, lbB, hB, f"{ph}b")
                    done_units.update(((lbA, hA), (lbB, hB)))
                    for lb in range(NLB):
                        if (lb, 2) in done_units and (lb, 0) in done_units and \
                           (lb, 1) in done_units and lb not in z_done:
                            z_done.add(lb)
                            for t_ in range(4):
                                pend_z.append(
                                    (lambda lb_=lb, t__=t_: z_tile(lb_, t__))
                                )
                    queue_ready_lns()
                    while ln_wait2:
                        bg_ops.extend(ln_chunk_ops(128 * ln_wait2.pop(0), 128))
                    ln_wait2.extend(ln_wait)
                    del ln_wait[:]

                z_done = set()
                for ph in range(6):
                    attn_phase(ph)
                while pend_z:
                    pend_z.pop(0)()
                while bg_ops:
                    bg_ops.pop(0)()
                queue_ready_lns()
                for s in ln_wait2 + ln_wait:
                    bg_ops.extend(ln_chunk_ops(128 * s, 128))
                del ln_wait2[:], ln_wait[:]
                while bg_ops:
                    bg_ops.pop(0)()

    nc.finalize()
    return nc


def _get_nc():
    if "nc" not in _CACHE:
        _CACHE["nc"] = _build()
    return _CACHE["nc"]


def build_in_maps(inputs):
    return _build_in_maps(**inputs)


F8NP = ml_dtypes.float8_e4m3fn


def _f8(x):
    return np.ascontiguousarray(np.asarray(x, dtype=np.float32).astype(F8NP))


def _bf(x):
    return np.ascontiguousarray(np.asarray(x, dtype=np.float32).astype(ml_dtypes.bfloat16))


def _pack_in(xT):
    # [D, N] -> [3, 128, 2, N] DR pairs
    return np.ascontiguousarray(
        xT.reshape(NPAIR, 2, 128, xT.shape[1]).transpose(0, 2, 1, 3)
    )


def _pack_w(wT):
    # [D, 192] -> [128, 3, 2, 192] DR pairs
    return np.ascontiguousarray(
        wT.reshape(NPAIR, 2, 128, HCOLS).transpose(2, 0, 1, 3)
    )


# psum partition order for the h0/h1 projection block + h2 block
_PERM = np.r_[0:32, 64:96, 32:64, 96:128, 128:160, 160:192]


def _build_in_maps(q, k, v, attention_mask, Wq, Wk, Wv, W, b, gamma, beta):
    q = np.asarray(q, dtype=np.float32)
    k = np.asarray(k, dtype=np.float32)
    v = np.asarray(v, dtype=np.float32)
    attention_mask = np.asarray(attention_mask, dtype=np.int32)
    Wq = np.asarray(Wq, dtype=np.float32)
    Wk = np.asarray(Wk, dtype=np.float32)
    Wv = np.asarray(Wv, dtype=np.float32)
    W = np.asarray(W, dtype=np.float32)
    b = np.asarray(b, dtype=np.float32)
    gamma = np.asarray(gamma, dtype=np.float32)
    beta = np.asarray(beta, dtype=np.float32)

    qT = [_pack_in(_f8(q[i].T)) for i in range(B)]
    kT = [_pack_in(_f8(k[i].T)) for i in range(B)]
    vT = [_pack_in(_f8(v[i].T)) for i in range(B)]
    maskT = [
        np.ascontiguousarray(attention_mask[i].reshape(SCH, 128).T) for i in range(B)
    ]
    gamma1 = np.ascontiguousarray(gamma.reshape(1, D))
    beta1 = np.ascontiguousarray(beta.reshape(1, D))

    in_maps = []
    for c in range(NCORES):
        bi, hg = c // 4, c % 4
        cs = slice(HCOLS * hg, HCOLS * (hg + 1))
        wqT = Wq[cs, :][_PERM].T * SQ  # [768, 192]
        wkT = Wk[cs, :][_PERM].T
        wvT = Wv[cs, :].T * VS
        wzT = W[:, cs].T * WS  # [192, 768]
        in_maps.append(
            {
                "qT8": qT[bi],
                "kT8": kT[bi],
                "vT8": vT[bi],
                "wq8": _pack_w(_f8(wqT)),
                "wk8": _pack_w(_f8(wkT)),
                "wv8": _pack_w(_f8(wvT)),
                "wz8": np.ascontiguousarray(
                    _f8(wzT).reshape(2, 96, D).transpose(1, 0, 2)
                ),
                "qres": _bf(
                    np.concatenate(
                        [q[bi, 512 * j + 128 * hg : 512 * j + 128 * (hg + 1), :]
                         for j in range(4)]
                    )
                    + b[None, :]
                ),
                "maskT": maskT[bi],
                "gamma1": gamma1,
                "beta1": beta1,
            }
        )
    return in_maps


def kernel(q, k, v, attention_mask, Wq, Wk, Wv, W, b, gamma, beta):
    nc = _get_nc()
    in_maps = _build_in_maps(q, k, v, attention_mask, Wq, Wk, Wv, W, b, gamma, beta)
    res = run_bass_kernel_spmd(nc, in_maps, core_ids=list(range(NCORES)))

    outp = np.empty((B, L, D), dtype=np.float32)
    for c in range(NCORES):
        bi, hg = c // 4, c % 4
        o = res.results[c]["out"]
        for j in range(4):
            outp[bi, 512 * j + 128 * hg : 512 * j + 128 * (hg + 1), :] = o[128 * j : 128 * (j + 1)]
    return outp


# revision 16
# speedup vs baseline: 1.4699x; 1.0267x over previous
"""Trainium2 Bass kernel for fused MultiHeadAttention + residual + LayerNorm.

Problem: B=2, L=S=2048, D=768, H=12 heads of dim 64, attention with key-padding
mask, output projection + bias, residual with q, LayerNorm(gamma, beta).

Sharding over 8 NeuronCores: data-parallel over batch (2 groups of 4 cores) x
tensor-parallel over heads (3 heads per core).

V3: full fp8e4m3 DoubleRow pipeline. All matmuls (QKV projections, QK^T
scores, attention-value, output projection) run in fp8 DoubleRow perf mode
(0.5 cycles/row, two 128-row k-tiles per pass = 4x fewer PE cycles vs bf16).
Q is pre-scaled by 1/ln2 in the wq weights so score PSUM arrives as
A8*(s/8); softmax exp splits between ScalarE (table exp, f8 out) and DVE
(one-shot Schraudolph writing e4m3 bits as int8). Denominator via a ones
column at V slot 64 -> PSUM partition 64; norm chain: ScalarE denom copy ->
DVE reciprocal_approx_fast -> Pool partition_broadcast -> DVE multiply.
V scaled x32 and W x8 so the f8 O/Z stores stay in normal range (Z carries
x256, removed in the LayerNorm residual add). ReduceScatter in 8 fp8 chunks;
LayerNorm on chunk pairs (Pool handles the elementwise stages, DVE the
bn_stats/rsqrt) with the last two chunks split for a short tail.
"""

import sys

sys.path.insert(0, "/opt/trn_rl_repo")

import ml_dtypes
import numpy as np

import concourse.bass as bass
import concourse.tile as tile
from concourse import bacc, mybir
from concourse.bass_utils import run_bass_kernel_spmd

F32 = mybir.dt.float32
BF16 = mybir.dt.bfloat16
F8 = mybir.dt.float8e4
I8 = mybir.dt.int8
I32 = mybir.dt.int32
DR = mybir.MatmulPerfMode.DoubleRow

D = 768
HD = 64
HPC = 3  # heads per core
HCOLS = HPC * HD  # 192
B, L, S = 2, 2048, 2048
NCORES = 8
GROUPS = [[0, 1, 2, 3], [4, 5, 6, 7]]
NPAIR = 3  # d-chunk DR pairs for projections (768 = 3 * 256)
SCH = S // 128  # 16 s-chunks
LB = 512
NLB = L // LB  # 4
NCHUNK = 8  # output chunk slots (64 rows/core); RS merged per l-block, split for the last
CROWS = L // NCHUNK  # 256
ORows = CROWS // 4  # 64 rows per core per chunk
LN_EPS = 1e-5
MASK_NEG = -1000000.0

# fp8 Schraudolph / scaling constants
A8 = 8.0 / np.log(2.0)  # e4m3 bits per e-fold
B8C = 7 * 8 - 0.46  # e4m3 exponent bias bits - Schraudolph C
M_SHIFT = 3.0  # global logit shift (softmax-invariant), keeps bits < 127
SQ = 1.0 / np.log(2.0)  # = A8/8, folded into wq
VS = 32.0  # V scale (keeps f8 O in normal range)
WS = 8.0  # W scale (keeps f8 W in normal range)
ZS = VS * WS  # Z carries x256

# exp engine split: True -> ScalarE table exp, False -> DVE Schraudolph
EXP_SCALAR = (True, True, True, False, True, False, True, False,
              True, False, True, False, True, False, True, False)  # 9 S / 7 D

_CACHE: dict = {}


def _build():
    nc = bacc.Bacc("TRN2", target_bir_lowering=False, debug=False, num_devices=NCORES)

    qT8 = nc.dram_tensor("qT8", [NPAIR, 128, 2, L], F8, kind="ExternalInput").ap()
    kT8 = nc.dram_tensor("kT8", [NPAIR, 128, 2, S], F8, kind="ExternalInput").ap()
    vT8 = nc.dram_tensor("vT8", [NPAIR, 128, 2, S], F8, kind="ExternalInput").ap()
    wq8 = nc.dram_tensor("wq8", [128, NPAIR, 2, HCOLS], F8, kind="ExternalInput").ap()
    wk8 = nc.dram_tensor("wk8", [128, NPAIR, 2, HCOLS], F8, kind="ExternalInput").ap()
    wv8 = nc.dram_tensor("wv8", [128, NPAIR, 2, HCOLS], F8, kind="ExternalInput").ap()
    wz8 = nc.dram_tensor("wz8", [96, 2, D], F8, kind="ExternalInput").ap()
    qres = nc.dram_tensor("qres", [512, D], BF16, kind="ExternalInput").ap()
    maskT = nc.dram_tensor("maskT", [128, SCH], I32, kind="ExternalInput").ap()
    gamma1 = nc.dram_tensor("gamma1", [1, D], F32, kind="ExternalInput").ap()
    beta1 = nc.dram_tensor("beta1", [1, D], F32, kind="ExternalInput").ap()
    out = nc.dram_tensor("out", [512, D], F32, kind="ExternalOutput").ap()

    AL = mybir.AluOpType
    ACT = mybir.ActivationFunctionType

    with tile.TileContext(nc, num_cores=NCORES) as tc:
        with (
            tc.tile_pool(name="persist", bufs=1) as pp,
            tc.tile_pool(name="dram", bufs=1, space="DRAM") as dram,
        ):
            kch = [pp.tile([128, 2, S], F8, name=f"kch{p}") for p in range(NPAIR)]
            vch = [pp.tile([128, 2, S], F8, name=f"vch{p}") for p in range(NPAIR)]
            qch = [pp.tile([128, 2, L], F8, name=f"qch{p}") for p in range(NPAIR)]
            wq_sb = pp.tile([128, NPAIR, 2, HCOLS], F8)
            wk_sb = pp.tile([128, NPAIR, 2, HCOLS], F8)
            wv_sb = pp.tile([128, NPAIR, 2, HCOLS], F8)
            wz_sb = pp.tile([96, 2, D], F8)
            KT1 = pp.tile([64, 2, S], F8)  # h0 on 0:32, h1 on 32:64; slot=lo/hi
            QT1 = pp.tile([64, 2, L], F8)
            KT2 = pp.tile([32, 2, S], F8)  # h2
            QT2 = pp.tile([32, 2, L], F8)
            V_sb = pp.tile([128, SCH // 2, HPC, 2, 128], F8)  # col 64 = ones, 65: pad
            OT = pp.tile([96, 2, L], F8)  # z contraction layout
            mask_i = pp.tile([128, SCH], I32)
            mask_f = pp.tile([128, SCH], F32)
            asc_bias = pp.tile([128, SCH], F32)
            dve_bias = pp.tile([128, SCH], F32)
            gam_b = pp.tile([128, D], F32)
            bet_b = pp.tile([128, D], F32)

            Z_dram = dram.tile([L, D], F8)
            Zr_dram = dram.tile([512, D], F8)
            sync_in = dram.tile([4, 192], BF16)
            sync_out = dram.tile([1, 192], BF16)

            # early dummy collective: absorbs inter-core rendezvous skew
            nc.gpsimd.collective_compute(
                "ReduceScatter",
                AL.add,
                replica_groups=GROUPS,
                ins=[sync_in[:, :].opt()],
                outs=[sync_out[:, :].opt()],
            )
            # weight / constant loads
            nc.sync.dma_start(out=wq_sb, in_=wq8[:, :, :, :])
            nc.sync.dma_start(out=wk_sb, in_=wk8[:, :, :, :])
            nc.sync.dma_start(out=wv_sb, in_=wv8[:, :, :, :])
            nc.sync.dma_start(out=wz_sb, in_=wz8[:, :, :])
            nc.sync.dma_start(out=mask_i, in_=maskT[:, :])
            nc.sync.dma_start(out=gam_b, in_=gamma1.to_broadcast([128, D]))
            nc.sync.dma_start(out=bet_b, in_=beta1.to_broadcast([128, D]))
            # input loads: k first (scores need full K), then q, then v
            for p in range(NPAIR):
                nc.sync.dma_start(out=kch[p], in_=kT8[p])
            for p in range(NPAIR):
                nc.sync.dma_start(out=qch[p], in_=qT8[p])
            for p in range(NPAIR):
                nc.sync.dma_start(out=vch[p], in_=vT8[p])

            nc.vector.tensor_copy(mask_f, mask_i)
            # ScalarE exp bias: (1-m)*MASK_NEG - M
            nc.scalar.activation(
                asc_bias, mask_f, ACT.Copy,
                bias=float(MASK_NEG - M_SHIFT), scale=-MASK_NEG,
            )
            # DVE Schraudolph bias: A8*((1-m)*MASK_NEG - M) + B8C
            nc.scalar.activation(
                dve_bias, mask_f, ACT.Copy,
                bias=float(A8 * (MASK_NEG - M_SHIFT) + B8C), scale=float(-A8 * MASK_NEG),
            )
            # ones column for softmax denominators (pad cols zeroed)
            nc.gpsimd.memset(V_sb[:, :, :, :, 64:128], 0.0)
            ones_t = pp.tile([128, SCH // 2, HPC, 2, 1], F8)
            nc.gpsimd.memset(ones_t, 1.0)
            nc.vector.tensor_copy(V_sb[:, :, :, :, 64:65], ones_t)

            # ---- Stage A: projections (fp8 DoubleRow) ----
            with tc.tile_pool(name="psp", bufs=1, space="PSUM") as psp:
                def proj01(dst, w_sb, src, lb, drain_eng):
                    lsl = slice(512 * lb, 512 * (lb + 1))
                    ps = psp.tile([128, 512], F32, tag="pA", bufs=3, name="p01")
                    for p in range(NPAIR):
                        nc.tensor.matmul(
                            ps, w_sb[:, p, :, 0:128], src[p][:, :, lsl],
                            start=(p == 0), stop=(p == NPAIR - 1), perf_mode=DR,
                        )
                    for t in range(2):
                        drain_eng(out=dst[:, t, lsl], in_=ps[64 * t : 64 * (t + 1)])

                def proj2(dst, w_sb, src, lb, drain_eng):
                    lsl = slice(512 * lb, 512 * (lb + 1))
                    ps = psp.tile([64, 512], F32, tag="pB", bufs=2, name="p2")
                    for p in range(NPAIR):
                        nc.tensor.matmul(
                            ps, w_sb[:, p, :, 128:192], src[p][:, :, lsl],
                            start=(p == 0), stop=(p == NPAIR - 1), perf_mode=DR,
                        )
                    for t in range(2):
                        drain_eng(out=dst[:, t, lsl], in_=ps[32 * t : 32 * (t + 1)])

                dve_cp = nc.vector.tensor_copy
                sc_cp = nc.scalar.copy

                # warm fillers keyed on the input DMA pairs keep the PE
                # clock ramped through the load window
                warm = pp.tile([128, 2, 512], F8, name="warm")
                nc.gpsimd.memset(warm, 0.0)
                wwi = [0]

                def warm_touch(ch, n):
                    for _ in range(n):
                        wp = psp.tile([128, 512], F32, tag="pA", bufs=3,
                                      name=f"wt{wwi[0]}")
                        wwi[0] += 1
                        nc.tensor.matmul(
                            wp, ch[:, :, 0:128], warm, start=True, stop=True,
                            perf_mode=DR,
                        )

                for p in range(NPAIR):
                    warm_touch(kch[p], 4)
                for lb in range(NLB):
                    proj01(KT1, wk_sb, kch, lb, dve_cp)
                for p in range(NPAIR):
                    warm_touch(qch[p], 3)
                proj01(QT1, wq_sb, qch, 0, dve_cp)

            # ---- Stage B: attention + out-projection + RS + LN ----
            # 12 units (head, lblock); 2 units per phase, lb-major so each
            # l-block completes (and its Z+RS can launch) as early as possible.
            UNITS = [(lb, h) for lb in range(NLB) for h in range(HPC)]
            PHASES = [(UNITS[2 * i], UNITS[2 * i + 1]) for i in range(6)]

            def kq_slices(lb, h, ssl):
                lsl = slice(512 * lb, 512 * (lb + 1))
                if h == 0:
                    return KT1[0:32, :, ssl], QT1[0:32, :, lsl]
                if h == 1:
                    return KT1[32:64, :, ssl], QT1[32:64, :, lsl]
                return KT2[0:32, :, ssl], QT2[0:32, :, lsl]

            with (
                tc.tile_pool(name="ptp", bufs=1) as ptp,
                tc.tile_pool(name="drp", bufs=1) as drp,
                tc.tile_pool(name="zsb", bufs=4) as zsb,
                tc.tile_pool(name="aps", bufs=1, space="PSUM") as aps,
                tc.tile_pool(name="ep", bufs=2) as ep,
            ):
                def norm_drain(oU, lb, h, nm):
                    # oU [128, 512]: rows 0:64 = O*VS, row 64 = denom, 65: junk
                    lsl = slice(512 * lb, 512 * (lb + 1))
                    dn = drp.tile([1, 512], F32, tag="dn", bufs=3, name=f"dn{nm}")
                    nc.scalar.copy(out=dn, in_=oU[64:65, :])
                    rr = drp.tile([1, 512], F32, tag="rr", bufs=3, name=f"rr{nm}")
                    nc.vector.reciprocal_approx_fast(rr, dn)
                    rb = drp.tile([64, 512], F32, tag="rb", bufs=3, name=f"rb{nm}")
                    nc.gpsimd.partition_broadcast(rb, rr)
                    if h == 0:
                        nc.vector.tensor_mul(OT[0:64, 0, lsl], oU[0:64, :], rb)
                    elif h == 1:
                        nc.vector.tensor_mul(OT[64:96, 0, lsl], oU[0:32, :], rb[0:32])
                        nc.vector.tensor_mul(OT[0:32, 1, lsl], oU[32:64, :], rb[32:64])
                    else:
                        nc.vector.tensor_mul(OT[32:64, 1, lsl], oU[0:32, :], rb[0:32])
                        nc.vector.tensor_mul(OT[64:96, 1, lsl], oU[32:64, :], rb[32:64])

                def ln_chunk_ops(off, rows):
                    # LayerNorm over one received RS slot as deferrable closures
                    zr = ep.tile([128, D], F8, name="zr")[0:rows]
                    qr = ep.tile([128, D], BF16, name="qr")[0:rows]
                    xb = ep.tile([128, D], F32, name="xb")[0:rows]
                    stats = ep.tile([128, 3, 6], F32, name="st")[0:rows]
                    mv = ep.tile([128, 2], F32, name="mv")[0:rows]
                    ve = ep.tile([128, 1], F32, name="ve")[0:rows]
                    sh = ep.tile([128, 1], I32, name="sh")[0:rows]
                    r0i = ep.tile([128, 1], I32, name="r0i")[0:rows]
                    t2 = ep.tile([128, 1], F32, name="t2")[0:rows]
                    rstd = ep.tile([128, 1], F32, name="rs")[0:rows]
                    t1 = ep.tile([128, D], F32, name="t1")[0:rows]
                    o = ep.tile([128, D], F32, name="o")[0:rows]

                    def s_dma():
                        nc.sync.dma_start(out=zr, in_=Zr_dram[off : off + rows])
                        nc.sync.dma_start(out=qr, in_=qres[off : off + rows])

                    def s_add():
                        nc.vector.scalar_tensor_tensor(
                            xb, zr, float(1.0 / ZS), qr, AL.mult, AL.add
                        )

                    def s_bn(g):
                        return lambda: nc.vector.bn_stats(
                            stats[:, g, :], xb[:, 256 * g : 256 * (g + 1)]
                        )

                    def s_aggr():
                        nc.vector.bn_aggr(mv, stats)
                        nc.vector.tensor_scalar_add(ve, mv[:, 1:2], float(LN_EPS))

                    def s_rsqrt():
                        nc.vector.tensor_scalar(
                            out=sh, in0=ve[:, 0:1].bitcast(I32), scalar1=1,
                            scalar2=None, op0=AL.arith_shift_right,
                        )
                        nc.vector.tensor_scalar(
                            out=r0i, in0=sh, scalar1=0x5F3759DF, scalar2=-1,
                            op0=AL.subtract, op1=AL.mult,
                        )
                        r0 = r0i[:, 0:1].bitcast(F32)
                        nc.vector.tensor_mul(t2, r0, r0)
                        nc.vector.tensor_mul(t2, t2, ve)
                        nc.vector.tensor_scalar(
                            out=t2, in0=t2, scalar1=-0.5, scalar2=1.5,
                            op0=AL.mult, op1=AL.add,
                        )
                        nc.vector.tensor_mul(rstd, r0, t2)

                    def s_stt1():
                        nc.vector.scalar_tensor_tensor(
                            t1, xb, mv[:, 0:1], gam_b[0:rows], AL.subtract, AL.mult
                        )

                    def s_stt2():
                        nc.vector.scalar_tensor_tensor(
                            o, t1, rstd, bet_b[0:rows], AL.mult, AL.add
                        )
                        nc.sync.dma_start(out=out[off : off + rows], in_=o)

                    return [s_dma, s_add, s_bn(0), s_bn(1), s_bn(2),
                            s_aggr, s_rsqrt, s_stt1, s_stt2]

                zi = [0]

                def z_tile(lb, t):
                    # Z = OT^T . wz (fp8 DR, contraction 192 = 2x96)
                    lt = 4 * lb + t
                    tsl = slice(128 * lt, 128 * (lt + 1))
                    zp = aps.tile([128, 1024], F32, tag="sw", bufs=3, name=f"zp{lt}")
                    nc.tensor.matmul(
                        zp[:, 0:512], OT[:, :, tsl], wz_sb[:, :, 0:512],
                        start=True, stop=True, perf_mode=DR,
                    )
                    nc.tensor.matmul(
                        zp[:, 512:768], OT[:, :, tsl], wz_sb[:, :, 512:768],
                        start=True, stop=True, perf_mode=DR,
                    )
                    zs = zsb.tile([128, D], F8, name="zs")
                    if zi[0] % 2 == 0:
                        nc.scalar.copy(out=zs, in_=zp[:, 0:768])
                    else:
                        nc.vector.tensor_copy(out=zs, in_=zp[:, 0:768])
                    zi[0] += 1
                    nc.sync.dma_start(out=Z_dram[tsl, :], in_=zs)
                    if lb < 3 and t == 3:
                        nc.gpsimd.collective_compute(
                            "ReduceScatter",
                            AL.add,
                            replica_groups=GROUPS,
                            ins=[Z_dram[512 * lb : 512 * (lb + 1), :].opt()],
                            outs=[Zr_dram[128 * lb : 128 * (lb + 1)].opt()],
                        )
                        pend_rs.add(lb)
                    elif lb == 3 and t == 3:
                        nc.gpsimd.collective_compute(
                            "ReduceScatter",
                            AL.add,
                            replica_groups=GROUPS,
                            ins=[Z_dram[1536:2048, :].opt()],
                            outs=[Zr_dram[384:512].opt()],
                        )
                        pend_rs.add(3)

                def projB01(dst, w_sb, src_, lb):
                    def go():
                        lsl = slice(512 * lb, 512 * (lb + 1))
                        ps = aps.tile([128, 1024], F32, tag="sw", bufs=3, name="pj")
                        for p in range(NPAIR):
                            nc.tensor.matmul(
                                ps[:, 0:512], w_sb[:, p, :, 0:128], src_[p][:, :, lsl],
                                start=(p == 0), stop=(p == NPAIR - 1), perf_mode=DR,
                            )
                        for t in range(2):
                            nc.vector.tensor_copy(
                                out=dst[:, t, lsl], in_=ps[64 * t : 64 * (t + 1), 0:512]
                            )
                    return go

                def projB2(dst, w_sb, src_, lb):
                    def go():
                        lsl = slice(512 * lb, 512 * (lb + 1))
                        ps = aps.tile([128, 1024], F32, tag="sw", bufs=3, name="pj2")
                        for p in range(NPAIR):
                            nc.tensor.matmul(
                                ps[0:64, 0:512], w_sb[:, p, :, 128:192],
                                src_[p][:, :, lsl],
                                start=(p == 0), stop=(p == NPAIR - 1), perf_mode=DR,
                            )
                        for t in range(2):
                            nc.scalar.copy(
                                out=dst[:, t, lsl], in_=ps[32 * t : 32 * (t + 1), 0:512]
                            )
                    return go

                def v_proj(sc):
                    def go():
                        ssl = slice(128 * sc, 128 * (sc + 1))
                        pv = aps.tile([128, 1024], F32, tag="sw", bufs=3, name="pv")
                        for p in range(NPAIR):
                            nc.tensor.matmul(
                                pv[:, 0:HCOLS], vch[p][:, :, ssl], wv_sb[:, p, :, :],
                                start=(p == 0), stop=(p == NPAIR - 1), perf_mode=DR,
                            )
                        nc.scalar.copy(
                            out=V_sb[:, sc // 2, :, sc % 2, 0:64],
                            in_=pv[:, 0:HCOLS].rearrange("p (h d) -> p h d", h=HPC),
                        )
                    return go

                pend_v = [v_proj(sc) for sc in range(SCH)]

                # remaining projections spread through early phases;
                # NEED[ph] = how many must have been emitted by end of phase ph
                pend_proj = [projB2(KT2, wk_sb, kch, lb) for lb in range(NLB)]
                pend_proj += [
                    projB2(QT2, wq_sb, qch, 0),
                    projB01(QT1, wq_sb, qch, 1),
                    projB2(QT2, wq_sb, qch, 1),
                    projB01(QT1, wq_sb, qch, 2),
                    projB2(QT2, wq_sb, qch, 2),
                    projB01(QT1, wq_sb, qch, 3),
                    projB2(QT2, wq_sb, qch, 3),
                ]
                proj_done = [0]
                PROJ_NEED = {0: 6, 1: 7, 2: 8, 3: 10, 4: 11, 5: 11}

                pend_rs = set()
                ln_queued = set()
                bg_ops = []
                pend_z = []
                ln_wait = []
                ln_wait2 = []
                done_units = set()

                def queue_ready_lns():
                    # defer one phase after the RS fires so in-order engine
                    # queues never head-of-line block on collective latency
                    for s in range(4):
                        if s not in ln_queued and s in pend_rs:
                            ln_queued.add(s)
                            ln_wait.append(s)

                def attn_phase(ph):
                    (lbA, hA), (lbB, hB) = PHASES[ph]
                    oA = aps.tile([128, 512], F32, tag="oA", bufs=1, name=f"oA{ph}")
                    oB = aps.tile([128, 512], F32, tag="oB", bufs=1, name=f"oB{ph}")
                    P = None
                    Ps = []

                    def emit_av(p):
                        nc.tensor.matmul(
                            oA, V_sb[:, p, hA, :, :], Ps[p][:, 0, :, :],
                            start=(p == 0), stop=(p == 7), perf_mode=DR,
                        )
                        nc.tensor.matmul(
                            oB, V_sb[:, p, hB, :, :], Ps[p][:, 1, :, :],
                            start=(p == 0), stop=(p == 7), perf_mode=DR,
                        )
                    for sc in range(SCH):
                        pair, par = sc // 2, sc % 2
                        ssl = slice(128 * sc, 128 * (sc + 1))
                        sw = aps.tile(
                            [128, 1024], F32, tag="sw", bufs=3, name=f"sw{ph}_{sc}"
                        )
                        kA, qA = kq_slices(lbA, hA, ssl)
                        kB, qB = kq_slices(lbB, hB, ssl)
                        nc.tensor.matmul(
                            sw[:, 0:512], kA, qA, start=True, stop=True, perf_mode=DR
                        )
                        nc.tensor.matmul(
                            sw[:, 512:1024], kB, qB, start=True, stop=True, perf_mode=DR
                        )
                        if par == 0:
                            P = ptp.tile(
                                [128, 2, 2, 512], F8, tag="p", bufs=4, name=f"P{ph}_{pair}"
                            )
                            Ps.append(P)
                        pdst = P[:, :, par, :]
                        if EXP_SCALAR[sc]:
                            nc.scalar.activation(
                                pdst, sw, ACT.Exp,
                                bias=asc_bias[:, sc : sc + 1], scale=float(1.0 / A8),
                            )
                        else:
                            nc.vector.tensor_scalar(
                                out=pdst.bitcast(I8), in0=sw,
                                scalar1=dve_bias[:, sc : sc + 1], scalar2=0.0,
                                op0=AL.add, op1=AL.max,
                            )
                        if par == 1 and pair >= 2:
                            emit_av(pair - 2)
                        if pend_v:
                            pend_v.pop(0)()
                            if pend_v:
                                pend_v.pop(0)()
                        elif par == 0 and pend_proj:
                            pend_proj.pop(0)()
                            proj_done[0] += 1
                        if par == 1 and pend_z:
                            pend_z.pop(0)()
                        if bg_ops:
                            bg_ops.pop(0)()
                    emit_av(6)
                    emit_av(7)
                    while pend_z:
                        pend_z.pop(0)()
                    while proj_done[0] < PROJ_NEED[ph] and pend_proj:
                        pend_proj.pop(0)()
                        proj_done[0] += 1
                    norm_drain(oA, lbA, hA, f"{ph}a")
                    norm_drain(oB, lbB, hB, f"{ph}b")
                    done_units.update(((lbA, hA), (lbB, hB)))
                    for lb in range(NLB):
                        if (lb, 2) in done_units and (lb, 0) in done_units and \
                           (lb, 1) in done_units and lb not in z_done:
                            z_done.add(lb)
                            for t_ in range(4):
                                pend_z.append(
                                    (lambda lb_=lb, t__=t_: z_tile(lb_, t__))
                                )
                    queue_ready_lns()
                    while ln_wait2:
                        bg_ops.extend(ln_chunk_ops(128 * ln_wait2.pop(0), 128))
                    ln_wait2.extend(ln_wait)
                    del ln_wait[:]

                z_done = set()
                for ph in range(6):
                    attn_phase(ph)
                while pend_z:
                    pend_z.pop(0)()
                while bg_ops:
                    bg_ops.pop(0)()
                queue_ready_lns()
                for s in ln_wait2 + ln_wait:
                    bg_ops.extend(ln_chunk_ops(128 * s, 128))
                del ln_wait2[:], ln_wait[:]
                while bg_ops:
                    bg_ops.pop(0)()

    nc.finalize()
    return nc


def _get_nc():
    if "nc" not in _CACHE:
        _CACHE["nc"] = _build()
    return _CACHE["nc"]


def build_in_maps(inputs):
    return _build_in_maps(**inputs)


F8NP = ml_dtypes.float8_e4m3fn


def _f8(x):
    return np.ascontiguousarray(np.asarray(x, dtype=np.float32).astype(F8NP))


def _bf(x):
    return np.ascontiguousarray(np.asarray(x, dtype=np.float32).astype(ml_dtypes.bfloat16))


def _pack_in(xT):
    # [D, N] -> [3, 128, 2, N] DR pairs
    return np.ascontiguousarray(
        xT.reshape(NPAIR, 2, 128, xT.shape[1]).transpose(0, 2, 1, 3)
    )


def _pack_w(wT):
    # [D, 192] -> [128, 3, 2, 192] DR pairs
    return np.ascontiguousarray(
        wT.reshape(NPAIR, 2, 128, HCOLS).transpose(2, 0, 1, 3)
    )


# psum partition order for the h0/h1 projection block + h2 block
_PERM = np.r_[0:32, 64:96, 32:64, 96:128, 128:160, 160:192]


def _build_in_maps(q, k, v, attention_mask, Wq, Wk, Wv, W, b, gamma, beta):
    q = np.asarray(q, dtype=np.float32)
    k = np.asarray(k, dtype=np.float32)
    v = np.asarray(v, dtype=np.float32)
    attention_mask = np.asarray(attention_mask, dtype=np.int32)
    Wq = np.asarray(Wq, dtype=np.float32)
    Wk = np.asarray(Wk, dtype=np.float32)
    Wv = np.asarray(Wv, dtype=np.float32)
    W = np.asarray(W, dtype=np.float32)
    b = np.asarray(b, dtype=np.float32)
    gamma = np.asarray(gamma, dtype=np.float32)
    beta = np.asarray(beta, dtype=np.float32)

    qT = [_pack_in(_f8(q[i].T)) for i in range(B)]
    kT = [_pack_in(_f8(k[i].T)) for i in range(B)]
    vT = [_pack_in(_f8(v[i].T)) for i in range(B)]
    maskT = [
        np.ascontiguousarray(attention_mask[i].reshape(SCH, 128).T) for i in range(B)
    ]
    gamma1 = np.ascontiguousarray(gamma.reshape(1, D))
    beta1 = np.ascontiguousarray(beta.reshape(1, D))

    in_maps = []
    for c in range(NCORES):
        bi, hg = c // 4, c % 4
        cs = slice(HCOLS * hg, HCOLS * (hg + 1))
        wqT = Wq[cs, :][_PERM].T * SQ  # [768, 192]
        wkT = Wk[cs, :][_PERM].T
        wvT = Wv[cs, :].T * VS
        wzT = W[:, cs].T * WS  # [192, 768]
        in_maps.append(
            {
                "qT8": qT[bi],
                "kT8": kT[bi],
                "vT8": vT[bi],
                "wq8": _pack_w(_f8(wqT)),
                "wk8": _pack_w(_f8(wkT)),
                "wv8": _pack_w(_f8(wvT)),
                "wz8": np.ascontiguousarray(
                    _f8(wzT).reshape(2, 96, D).transpose(1, 0, 2)
                ),
                "qres": _bf(
                    np.concatenate(
                        [q[bi, 512 * j + 128 * hg : 512 * j + 128 * (hg + 1), :]
                         for j in range(4)]
                    )
                    + b[None, :]
                ),
                "maskT": maskT[bi],
                "gamma1": gamma1,
                "beta1": beta1,
            }
        )
    return in_maps


def kernel(q, k, v, attention_mask, Wq, Wk, Wv, W, b, gamma, beta):
    nc = _get_nc()
    in_maps = _build_in_maps(q, k, v, attention_mask, Wq, Wk, Wv, W, b, gamma, beta)
    res = run_bass_kernel_spmd(nc, in_maps, core_ids=list(range(NCORES)))

    outp = np.empty((B, L, D), dtype=np.float32)
    for c in range(NCORES):
        bi, hg = c // 4, c % 4
        o = res.results[c]["out"]
        for j in range(4):
            outp[bi, 512 * j + 128 * hg : 512 * j + 128 * (hg + 1), :] = o[128 * j : 128 * (j + 1)]
    return outp
